# revision 5
# baseline (speedup 1.0000x reference)
"""DepLabeledGCN Trainium2 kernel.

Math (per batch b):
    for 2 layers:  msg = sum_l A_l @ h @ W_l^T ;  h = relu(msg / denom)
    where A_l[i,j] = adj[i,j] * (lab[i,j] == l)   (layer-independent masks)
    then 2-layer MLP with relu.

Restructured "aggregation-first":
    sT chunk (l,kc): s_l^T[kc] = (h[:, kc-chunk])^T-style matmul vs mask
    msg = sum_{l,kc} sT[l,kc] (as lhsT) @ W_l^T[kc]   (PSUM accumulation)

Sharding: label-parallel across 8 cores (6 labels each, weights SBUF-resident),
batch-chunked AllReduce (3+5) of partial msg after layer 1 pipelined with
compute, ReduceScatter after layer 2 (each core receives its own summed batch),
then per-core MLP on its own batch.  Matmuls fp16 (masks exact 0/1, PSUM
accumulation fp32), collectives fp16.
"""

import sys

if '/opt/trn_rl_repo' not in sys.path:
    sys.path.insert(0, '/opt/trn_rl_repo')

import numpy as np

B, N, D, L = 8, 128, 512, 48
NCORES = 8
LC = L // NCORES          # labels per core
KC = D // 128             # 128-wide k chunks
NUM_LAYERS = 2
C1 = 3                    # batches in first layer-1 AllReduce chunk
C2 = B - C1
N_WARM = 14               # keep-PE-warm dummy matmuls during ReduceScatter

_CACHE = {}


def _build_nc():
    import concourse.bass as bass
    import concourse.mybir as mybir
    import concourse.tile as tile
    from concourse import bacc
    from concourse.masks import make_identity

    dt = mybir.dt
    f32 = dt.float32
    f16 = dt.float16
    Alu = mybir.AluOpType

    nc = bacc.Bacc("TRN2", target_bir_lowering=False, debug=False,
                   num_devices=NCORES)

    gcn_e = nc.dram_tensor("gcn", [N, B, D], f32, kind="ExternalInput").ap()
    adjT_e = nc.dram_tensor("adjT", [N, B, N], f32, kind="ExternalInput").ap()
    labT_e = nc.dram_tensor("labT", [N, B, N], f32, kind="ExternalInput").ap()
    adjR_e = nc.dram_tensor("adjR", [N, B, N], f32, kind="ExternalInput").ap()
    adjown_e = nc.dram_tensor("adjown", [N, N], f32, kind="ExternalInput").ap()
    wT_e = nc.dram_tensor("wT", [128, LC, KC, D], f16, kind="ExternalInput").ap()
    w0T_e = nc.dram_tensor("w0T", [128, KC, D], f16, kind="ExternalInput").ap()
    w1T_e = nc.dram_tensor("w1T", [128, KC, D], f16, kind="ExternalInput").ap()
    b0_e = nc.dram_tensor("b0", [128, KC], f32, kind="ExternalInput").ap()
    b1_e = nc.dram_tensor("b1", [128, KC], f32, kind="ExternalInput").ap()
    loff_e = nc.dram_tensor("loff", [128, LC], f32, kind="ExternalInput").ap()
    out_e = nc.dram_tensor("out", [KC, 128, N], f32, kind="ExternalOutput").ap()

    with tile.TileContext(nc) as tc:
        with (
            tc.tile_pool(name="const", bufs=1) as cpool,
            tc.tile_pool(name="stage", bufs=3) as stage_pool,
            tc.tile_pool(name="sT", bufs=2) as sT_pool,
            tc.tile_pool(name="msg", bufs=2) as msg_pool,
            tc.tile_pool(name="spsum", bufs=3, space="PSUM") as spsum,
            tc.tile_pool(name="mpsum", bufs=2, space="PSUM") as mpsum,
            tc.tile_pool(name="dram", bufs=1, space="DRAM") as dram,
        ):
            # -------- critical-path input loads (order = DMA priority) ------
            adjT_sb = cpool.tile([128, B, N], f32, tag="adjT")
            nc.sync.dma_start(adjT_sb[:], adjT_e)
            labT_sb = cpool.tile([128, B, N], f32, tag="labT")
            nc.sync.dma_start(labT_sb[:], labT_e)
            loff_sb = cpool.tile([128, LC], f32, tag="loff")
            nc.sync.dma_start(loff_sb[:], loff_e)

            # layer-0 h: load gcn f32, cast to fp16 on ScalarE (Copy only)
            h_lo = [cpool.tile([128, C1, D], f16, tag=f"h{ly}_lo",
                               name=f"h{ly}_lo") for ly in range(NUM_LAYERS)]
            h_hi = [cpool.tile([128, C2, D], f16, tag=f"h{ly}_hi",
                               name=f"h{ly}_hi") for ly in range(NUM_LAYERS)]
            gcn_sb = cpool.tile([128, B, D], f32, tag="gcn_sb")
            nc.sync.dma_start(gcn_sb[:, :C1], gcn_e[:, :C1])
            nc.scalar.copy(h_lo[0][:], gcn_sb[:, :C1])
            nc.sync.dma_start(gcn_sb[:, C1:], gcn_e[:, C1:])
            nc.scalar.copy(h_hi[0][:], gcn_sb[:, C1:])

            # per-label weight loads so msum(l=0) can start early
            wT_sb = cpool.tile([128, LC, KC, D], f16, tag="wT")
            for l in range(LC):
                nc.sync.dma_start(wT_sb[:, l], wT_e[:, l])

            # -------- masks: maskT[j, b, l, i] = (labT==loff[l]) * adjT -----
            maskT = cpool.tile([128, B, LC, N], f16, tag="maskT")
            for l in range(LC):
                nc.vector.scalar_tensor_tensor(
                    out=maskT[:, :, l, :],
                    in0=labT_sb[:],
                    scalar=loff_sb[:, l:l + 1],
                    in1=adjT_sb[:],
                    op0=Alu.is_equal,
                    op1=Alu.mult,
                )

            # -------- non-critical loads ------------------------------------
            adjR_sb = cpool.tile([128, B, N], f32, tag="adjR")
            nc.sync.dma_start(adjR_sb[:], adjR_e)
            adjown_sb = cpool.tile([128, N], f32, tag="adjown")
            nc.sync.dma_start(adjown_sb[:], adjown_e)
            b0_sb = cpool.tile([128, KC], f32, tag="b0")
            nc.sync.dma_start(b0_sb[:], b0_e)
            b1_sb = cpool.tile([128, KC], f32, tag="b1")
            nc.sync.dma_start(b1_sb[:], b1_e)
            w0T_sb = cpool.tile([128, KC, D], f16, tag="w0T")
            nc.sync.dma_start(w0T_sb[:], w0T_e)
            w1T_sb = cpool.tile([128, KC, D], f16, tag="w1T")
            nc.sync.dma_start(w1T_sb[:], w1T_e)

            identity = cpool.tile([128, 128], f16, tag="ident")
            make_identity(nc, identity[:])
            warm_sb = cpool.tile([128, D], f16, tag="warmsb")
            nc.gpsimd.memset(warm_sb[:], 0.0)

            # -------- denominators (needed first at ~50us) ------------------
            den = cpool.tile([128, B], f32, tag="den")
            nc.vector.tensor_reduce(den[:], adjR_sb[:], mybir.AxisListType.X,
                                    Alu.add)
            nc.vector.tensor_scalar_add(den[:], den[:], 1.0)
            recip = cpool.tile([128, B], f32, tag="recip")
            nc.vector.reciprocal(recip[:], den[:])

            den_o = cpool.tile([128, 1], f32, tag="deno")
            nc.vector.tensor_reduce(den_o[:], adjown_sb[:],
                                    mybir.AxisListType.X, Alu.add)
            nc.vector.tensor_scalar_add(den_o[:], den_o[:], 1.0)
            recip_o = cpool.tile([128, 1], f32, tag="recipo")
            nc.vector.reciprocal(recip_o[:], den_o[:])

            # -------- collective DRAM buffers -------------------------------
            m1in = [dram.tile([N, C1, D], f16, name="m1in0", tag="m1in0"),
                    dram.tile([N, C2, D], f16, name="m1in1", tag="m1in1")]
            m1out = [dram.tile([N, C1, D], f16, name="m1out0", tag="m1out0"),
                     dram.tile([N, C2, D], f16, name="m1out1", tag="m1out1")]
            m2in = dram.tile([B, N, D], f16, tag="m2in")
            m2own = dram.tile([1, N, D], f16, tag="m2own")

            rg = [list(range(NCORES))]

            # -------- GCN layers --------------------------------------------
            def emit_s(ly, b):
                """s-phase for batch b: 3 'thirds' of 2 labels each."""
                if b < C1:
                    h_src, bi = h_lo[ly], b
                else:
                    h_src, bi = h_hi[ly], b - C1
                sT = sT_pool.tile([128, LC, KC, 128], f16, tag="sT", name="sT")
                for t in range(3):
                    ps = spsum.tile([128, 2, KC, 128], f32, tag="spsum",
                                    name="spsum")
                    for kc in range(KC):
                        for l2 in range(2):
                            l = 2 * t + l2
                            nc.tensor.matmul(
                                ps[:, l2, kc, :],
                                lhsT=h_src[:, bi, kc * 128:(kc + 1) * 128],
                                rhs=maskT[:, b, l, :],
                                start=True, stop=True,
                            )
                    # copy/cast psum -> sbuf sT (DVE for t=0,2; ACT Copy t=1)
                    if t == 1:
                        nc.scalar.copy(sT[:, 2 * t:2 * t + 2], ps[:])
                    else:
                        nc.vector.tensor_copy(sT[:, 2 * t:2 * t + 2], ps[:])
                return sT

            def emit_msum(ly, b, sT):
                pm = mpsum.tile([128, D], f32, tag="mm", name="mm")
                n_mm = LC * KC
                i = 0
                for l in range(LC):
                    for kc in range(KC):
                        nc.tensor.matmul(
                            pm[:],
                            lhsT=sT[:, l, kc, :],
                            rhs=wT_sb[:, l, kc, :],
                            start=(i == 0), stop=(i == n_mm - 1),
                        )
                        i += 1
                msg = msg_pool.tile([128, D], f16, tag="msg", name="msg")
                nc.vector.tensor_copy(msg[:], pm[:])
                if ly == 0:
                    if b < C1:
                        nc.sync.dma_start(m1in[0][:, b], msg[:])
                    else:
                        nc.sync.dma_start(m1in[1][:, b - C1], msg[:])
                else:
                    nc.sync.dma_start(m2in[b], msg[:])

            def emit_readback(chunk):
                """AR output -> relu(msg * recip) -> next-layer h (on DVE)."""
                nb = C1 if chunk == 0 else C2
                boff = 0 if chunk == 0 else C1
                mrb = stage_pool.tile([128, nb, D], f16, tag=f"mrb{chunk}",
                                      name=f"mrb{chunk}")
                nc.sync.dma_start(mrb[:], m1out[chunk][:])
                dst = h_lo[1] if chunk == 0 else h_hi[1]
                for bb in range(nb):
                    nc.vector.tensor_scalar(
                        dst[:, bb, :], mrb[:, bb, :],
                        recip[:, boff + bb:boff + bb + 1], 0.0,
                        Alu.mult, Alu.max)

            for ly in range(NUM_LAYERS):
                sT_prev = emit_s(ly, 0)
                for b in range(B):
                    sT_next = emit_s(ly, b + 1) if b + 1 < B else None
                    emit_msum(ly, b, sT_prev)
                    sT_prev = sT_next
                    if ly == 0 and b == C1 - 1:
                        nc.gpsimd.collective_compute(
                            "AllReduce", Alu.add, replica_groups=rg,
                            ins=[m1in[0].opt()], outs=[m1out[0].opt()])
                        emit_readback(0)
                if ly == 0:
                    nc.gpsimd.collective_compute(
                        "AllReduce", Alu.add, replica_groups=rg,
                        ins=[m1in[1].opt()], outs=[m1out[1].opt()])
                    emit_readback(1)

            # layer 2: ReduceScatter -> each core gets its own batch summed
            nc.gpsimd.collective_compute(
                "ReduceScatter", Alu.add, replica_groups=rg,
                ins=[m2in.opt()], outs=[m2own.opt()])

            # keep PE warm during the ReduceScatter (HAM would re-throttle)
            for i in range(N_WARM):
                pw = mpsum.tile([128, D], f32, tag="mm", name="warmp")
                for j in range(2):
                    nc.tensor.matmul(pw[:], lhsT=identity[:], rhs=warm_sb[:],
                                     start=(j == 0), stop=(j == 1))
                nc.vector.tensor_copy(warm_sb[:, :16], pw[:, :16])

            mo = cpool.tile([128, D], f16, tag="mo")
            nc.sync.dma_start(mo[:], m2own[0])
            h_own = cpool.tile([128, D], f16, tag="hown")
            nc.vector.tensor_scalar(h_own[:], mo[:], recip_o[:], 0.0,
                                    Alu.mult, Alu.max)

            # -------- MLP on own batch --------------------------------------
            hT = cpool.tile([128, KC, 128], f16, tag="hT")
            pt = mpsum.tile([128, KC, 128], f16, tag="mm", name="ptr")
            for kc in range(KC):
                nc.tensor.transpose(pt[:, kc, :],
                                    h_own[:, kc * 128:(kc + 1) * 128],
                                    identity[:])
            nc.vector.tensor_copy(hT[:], pt[:])

            x1T = cpool.tile([128, KC, 128], f16, tag="x1T")
            px1 = mpsum.tile([128, KC, 128], f32, tag="mm", name="px1")
            for blk in range(KC):
                for kc in range(KC):
                    nc.tensor.matmul(
                        px1[:, blk, :],
                        lhsT=w0T_sb[:, kc, blk * 128:(blk + 1) * 128],
                        rhs=hT[:, kc, :],
                        start=(kc == 0), stop=(kc == KC - 1),
                    )
            for blk in range(KC):
                nc.vector.tensor_scalar(x1T[:, blk, :], px1[:, blk, :],
                                        b0_sb[:, blk:blk + 1], 0.0,
                                        Alu.add, Alu.max)

            x2 = cpool.tile([128, KC, 128], f32, tag="x2")
            px2 = mpsum.tile([128, KC, 128], f32, tag="mm", name="px2")
            for blk in range(KC):
                for kc in range(KC):
                    nc.tensor.matmul(
                        px2[:, blk, :],
                        lhsT=w1T_sb[:, kc, blk * 128:(blk + 1) * 128],
                        rhs=x1T[:, kc, :],
                        start=(kc == 0), stop=(kc == KC - 1),
                    )
            for blk in range(KC):
                nc.vector.tensor_scalar(x2[:, blk, :], px2[:, blk, :],
                                        b1_sb[:, blk:blk + 1], 0.0,
                                        Alu.add, Alu.max)

            for blk in range(KC):
                nc.sync.dma_start(out_e[blk], x2[:, blk, :])

    nc.compile()
    return nc


def _get_nc():
    if "nc" not in _CACHE:
        _CACHE["nc"] = _build_nc()
    return _CACHE["nc"]


def kernel(gcn_inputs, word_seq_len, adj_matrix, dep_label_matrix,
           w_params, mlp_w0, mlp_b0, mlp_w1, mlp_b1, **_unused):
    from concourse.bass_utils import run_bass_kernel_spmd

    gcn = np.asarray(gcn_inputs, dtype=np.float32)
    adj = np.asarray(adj_matrix, dtype=np.float32)
    lab = np.asarray(dep_label_matrix)
    w = np.asarray(w_params, dtype=np.float32)
    w0 = np.asarray(mlp_w0, dtype=np.float32)
    w1 = np.asarray(mlp_w1, dtype=np.float32)
    b0 = np.asarray(mlp_b0, dtype=np.float32)
    b1 = np.asarray(mlp_b1, dtype=np.float32)

    # [j, b, i] layouts for SBUF partition-major contiguous DMA
    adjT = np.ascontiguousarray(adj.transpose(2, 0, 1))
    labT = np.ascontiguousarray(lab.transpose(2, 0, 1)).astype(np.float32)
    adjR = np.ascontiguousarray(adj.transpose(1, 0, 2))
    gcnT = np.ascontiguousarray(gcn.transpose(1, 0, 2))
    # wT[kmod, l, kc, d] = w[l, d, kc*128+kmod]
    wT = w.transpose(0, 2, 1).reshape(L, KC, 128, D).transpose(2, 0, 1, 3)
    wT = np.ascontiguousarray(wT).astype(np.float16)
    w0T = np.ascontiguousarray(
        w0.T.reshape(KC, 128, D).transpose(1, 0, 2)).astype(np.float16)
    w1T = np.ascontiguousarray(
        w1.T.reshape(KC, 128, D).transpose(1, 0, 2)).astype(np.float16)
    b0r = np.ascontiguousarray(b0.reshape(KC, 128).T)   # [dmod, dblk]
    b1r = np.ascontiguousarray(b1.reshape(KC, 128).T)

    in_maps = []
    for c in range(NCORES):
        loff = np.tile(np.arange(LC * c, LC * (c + 1), dtype=np.float32),
                       (128, 1))
        in_maps.append({
            "gcn": gcnT,
            "adjT": adjT,
            "labT": labT,
            "adjR": adjR,
            "adjown": np.ascontiguousarray(adj[c]),
            "wT": np.ascontiguousarray(wT[:, LC * c:LC * (c + 1)]),
            "w0T": w0T,
            "w1T": w1T,
            "b0": b0r,
            "b1": b1r,
            "loff": loff,
        })

    nc = _get_nc()
    res = run_bass_kernel_spmd(nc, in_maps, list(range(NCORES)))

    out = np.empty((B, N, D), dtype=np.float32)
    for c in range(NCORES):
        arr = res.results[c]["out"]          # [dblk, dmod, i]
        out[c] = np.transpose(arr, (2, 0, 1)).reshape(N, D)
    return out


# revision 6
# speedup vs baseline: 1.0185x; 1.0185x over previous
"""DepLabeledGCN Trainium2 kernel.

Math (per batch b):
    for 2 layers:  msg = sum_l A_l @ h @ W_l^T ;  h = relu(msg / denom)
    where A_l[i,j] = adj[i,j] * (lab[i,j] == l)   (layer-independent masks)
    then 2-layer MLP with relu.

Restructured "aggregation-first":
    sT chunk (l,kc): s_l^T[kc] = (h[:, kc-chunk])^T-style matmul vs mask
    msg = sum_{l,kc} sT[l,kc] (as lhsT) @ W_l^T[kc]   (PSUM accumulation)

Sharding: label-parallel across 8 cores (6 labels each, weights SBUF-resident),
batch-chunked AllReduce (3+5) of partial msg after layer 1 pipelined with
compute, ReduceScatter after layer 2 (each core receives its own summed batch),
then per-core MLP on its own batch.  Matmuls fp16 (masks exact 0/1, PSUM
accumulation fp32), collectives fp16.
"""

import sys

if '/opt/trn_rl_repo' not in sys.path:
    sys.path.insert(0, '/opt/trn_rl_repo')

import numpy as np

B, N, D, L = 8, 128, 512, 48
NCORES = 8
LC = L // NCORES          # labels per core
KC = D // 128             # 128-wide k chunks
NUM_LAYERS = 2
C1 = 3                    # batches in first layer-1 AllReduce chunk
C2 = B - C1
N_WARM = 20               # keep-PE-warm dummy matmuls during ReduceScatter

_CACHE = {}


def _build_nc():
    import concourse.bass as bass
    import concourse.mybir as mybir
    import concourse.tile as tile
    from concourse import bacc
    from concourse.masks import make_identity

    dt = mybir.dt
    f32 = dt.float32
    f16 = dt.float16
    Alu = mybir.AluOpType

    nc = bacc.Bacc("TRN2", target_bir_lowering=False, debug=False,
                   num_devices=NCORES)

    gcn_e = nc.dram_tensor("gcn", [N, B, D], f32, kind="ExternalInput").ap()
    adjT_e = nc.dram_tensor("adjT", [N, B, N], f32, kind="ExternalInput").ap()
    labT_e = nc.dram_tensor("labT", [N, B, N], f32, kind="ExternalInput").ap()
    adjR_e = nc.dram_tensor("adjR", [N, B, N], f32, kind="ExternalInput").ap()
    adjown_e = nc.dram_tensor("adjown", [N, N], f32, kind="ExternalInput").ap()
    wT_e = nc.dram_tensor("wT", [128, LC, KC, D], f16, kind="ExternalInput").ap()
    w0T_e = nc.dram_tensor("w0T", [128, KC, D], f16, kind="ExternalInput").ap()
    w1T_e = nc.dram_tensor("w1T", [128, KC, D], f16, kind="ExternalInput").ap()
    b0_e = nc.dram_tensor("b0", [128, KC], f32, kind="ExternalInput").ap()
    b1_e = nc.dram_tensor("b1", [128, KC], f32, kind="ExternalInput").ap()
    loff_e = nc.dram_tensor("loff", [128, LC], f32, kind="ExternalInput").ap()
    out_e = nc.dram_tensor("out", [KC, 128, N], f32, kind="ExternalOutput").ap()

    with tile.TileContext(nc) as tc:
        with (
            tc.tile_pool(name="const", bufs=1) as cpool,
            tc.tile_pool(name="stage", bufs=3) as stage_pool,
            tc.tile_pool(name="sT", bufs=2) as sT_pool,
            tc.tile_pool(name="msg", bufs=2) as msg_pool,
            tc.tile_pool(name="spsum", bufs=3, space="PSUM") as spsum,
            tc.tile_pool(name="mpsum", bufs=2, space="PSUM") as mpsum,
            tc.tile_pool(name="dram", bufs=1, space="DRAM") as dram,
        ):
            rgw = [list(range(NCORES))]
            # tiny dummy AllReduce first: absorbs the NRT/ncfw collective
            # init + kernel-entry barrier concurrently with layer-1 compute,
            # so the first real AllReduce starts promptly when triggered.
            dum_in = dram.tile([1, 16], f32, tag="dumin")
            dum_out = dram.tile([1, 16], f32, tag="dumout")
            nc.gpsimd.collective_compute(
                "AllReduce", mybir.AluOpType.add, replica_groups=rgw,
                ins=[dum_in.opt()], outs=[dum_out.opt()])

            # -------- critical-path input loads (order = DMA priority) ------
            adjT_sb = cpool.tile([128, B, N], f32, tag="adjT")
            nc.sync.dma_start(adjT_sb[:, :1], adjT_e[:, :1])
            labT_sb = cpool.tile([128, B, N], f32, tag="labT")
            nc.sync.dma_start(labT_sb[:, :1], labT_e[:, :1])
            loff_sb = cpool.tile([128, LC], f32, tag="loff")
            nc.sync.dma_start(loff_sb[:], loff_e)

            # layer-0 h: load gcn f32, cast to fp16 on ScalarE (Copy only)
            h_lo = [cpool.tile([128, C1, D], f16, tag=f"h{ly}_lo",
                               name=f"h{ly}_lo") for ly in range(NUM_LAYERS)]
            h_hi = [cpool.tile([128, C2, D], f16, tag=f"h{ly}_hi",
                               name=f"h{ly}_hi") for ly in range(NUM_LAYERS)]
            gcn_sb = cpool.tile([128, B, D], f32, tag="gcn_sb")
            nc.sync.dma_start(gcn_sb[:, :1], gcn_e[:, :1])
            nc.scalar.copy(h_lo[0][:, :1], gcn_sb[:, :1])

            # per-label weight loads so msum(l=0) can start early
            wT_sb = cpool.tile([128, LC, KC, D], f16, tag="wT")
            for l in range(LC):
                nc.sync.dma_start(wT_sb[:, l], wT_e[:, l])

            # -------- masks: maskT[j, b, l, i] = (labT==loff[l]) * adjT -----
            # b0 pass first so s(b0) can start as soon as possible
            maskT = cpool.tile([128, B, LC, N], f16, tag="maskT")
            for l in range(LC):
                nc.vector.scalar_tensor_tensor(
                    out=maskT[:, :1, l, :],
                    in0=labT_sb[:, :1],
                    scalar=loff_sb[:, l:l + 1],
                    in1=adjT_sb[:, :1],
                    op0=Alu.is_equal,
                    op1=Alu.mult,
                )
            # rest of the inputs + masks
            nc.sync.dma_start(adjT_sb[:, 1:], adjT_e[:, 1:])
            nc.sync.dma_start(labT_sb[:, 1:], labT_e[:, 1:])
            nc.sync.dma_start(gcn_sb[:, 1:C1], gcn_e[:, 1:C1])
            nc.scalar.copy(h_lo[0][:, 1:], gcn_sb[:, 1:C1])
            nc.sync.dma_start(gcn_sb[:, C1:], gcn_e[:, C1:])
            nc.scalar.copy(h_hi[0][:], gcn_sb[:, C1:])
            for l in range(LC):
                nc.vector.scalar_tensor_tensor(
                    out=maskT[:, 1:, l, :],
                    in0=labT_sb[:, 1:],
                    scalar=loff_sb[:, l:l + 1],
                    in1=adjT_sb[:, 1:],
                    op0=Alu.is_equal,
                    op1=Alu.mult,
                )

            # -------- non-critical loads ------------------------------------
            adjR_sb = cpool.tile([128, B, N], f32, tag="adjR")
            nc.sync.dma_start(adjR_sb[:], adjR_e)
            adjown_sb = cpool.tile([128, N], f32, tag="adjown")
            nc.sync.dma_start(adjown_sb[:], adjown_e)
            b0_sb = cpool.tile([128, KC], f32, tag="b0")
            nc.sync.dma_start(b0_sb[:], b0_e)
            b1_sb = cpool.tile([128, KC], f32, tag="b1")
            nc.sync.dma_start(b1_sb[:], b1_e)
            w0T_sb = cpool.tile([128, KC, D], f16, tag="w0T")
            nc.sync.dma_start(w0T_sb[:], w0T_e)
            w1T_sb = cpool.tile([128, KC, D], f16, tag="w1T")
            nc.sync.dma_start(w1T_sb[:], w1T_e)

            identity = cpool.tile([128, 128], f16, tag="ident")
            make_identity(nc, identity[:])
            warm_sb = cpool.tile([128, D], f16, tag="warmsb")
            nc.gpsimd.memset(warm_sb[:], 0.0)

            # -------- denominators (needed first at ~50us) ------------------
            den = cpool.tile([128, B], f32, tag="den")
            nc.vector.tensor_reduce(den[:], adjR_sb[:], mybir.AxisListType.X,
                                    Alu.add)
            nc.vector.tensor_scalar_add(den[:], den[:], 1.0)
            recip = cpool.tile([128, B], f32, tag="recip")
            nc.vector.reciprocal(recip[:], den[:])

            den_o = cpool.tile([128, 1], f32, tag="deno")
            nc.vector.tensor_reduce(den_o[:], adjown_sb[:],
                                    mybir.AxisListType.X, Alu.add)
            nc.vector.tensor_scalar_add(den_o[:], den_o[:], 1.0)
            recip_o = cpool.tile([128, 1], f32, tag="recipo")
            nc.vector.reciprocal(recip_o[:], den_o[:])

            # -------- collective DRAM buffers -------------------------------
            m1in = [dram.tile([N, C1, D], f16, name="m1in0", tag="m1in0"),
                    dram.tile([N, C2, D], f16, name="m1in1", tag="m1in1")]
            m1out = [dram.tile([N, C1, D], f16, name="m1out0", tag="m1out0"),
                     dram.tile([N, C2, D], f16, name="m1out1", tag="m1out1")]
            m2in = dram.tile([B, N, D], f16, tag="m2in")
            m2own = dram.tile([1, N, D], f16, tag="m2own")

            rg = rgw

            # -------- GCN layers --------------------------------------------
            def emit_s(ly, b):
                """s-phase for batch b: 3 'thirds' of 2 labels each."""
                if b < C1:
                    h_src, bi = h_lo[ly], b
                else:
                    h_src, bi = h_hi[ly], b - C1
                sT = sT_pool.tile([128, LC, KC, 128], f16, tag="sT", name="sT")
                for t in range(3):
                    ps = spsum.tile([128, 2, KC, 128], f32, tag="spsum",
                                    name="spsum")
                    for kc in range(KC):
                        for l2 in range(2):
                            l = 2 * t + l2
                            nc.tensor.matmul(
                                ps[:, l2, kc, :],
                                lhsT=h_src[:, bi, kc * 128:(kc + 1) * 128],
                                rhs=maskT[:, b, l, :],
                                start=True, stop=True,
                            )
                    # copy/cast psum -> sbuf sT (DVE for t=0,2; ACT Copy t=1)
                    if t == 1:
                        nc.scalar.copy(sT[:, 2 * t:2 * t + 2], ps[:])
                    else:
                        nc.vector.tensor_copy(sT[:, 2 * t:2 * t + 2], ps[:])
                return sT

            def emit_msum(ly, b, sT):
                pm = mpsum.tile([128, D], f32, tag="mm", name="mm")
                n_mm = LC * KC
                i = 0
                for l in range(LC):
                    for kc in range(KC):
                        nc.tensor.matmul(
                            pm[:],
                            lhsT=sT[:, l, kc, :],
                            rhs=wT_sb[:, l, kc, :],
                            start=(i == 0), stop=(i == n_mm - 1),
                        )
                        i += 1
                msg = msg_pool.tile([128, D], f16, tag="msg", name="msg")
                nc.vector.tensor_copy(msg[:], pm[:])
                if ly == 0:
                    if b < C1:
                        nc.sync.dma_start(m1in[0][:, b], msg[:])
                    else:
                        nc.sync.dma_start(m1in[1][:, b - C1], msg[:])
                else:
                    nc.sync.dma_start(m2in[b], msg[:])

            def emit_readback(chunk):
                """AR output -> relu(msg * recip) -> next-layer h (on DVE)."""
                nb = C1 if chunk == 0 else C2
                boff = 0 if chunk == 0 else C1
                mrb = stage_pool.tile([128, nb, D], f16, tag=f"mrb{chunk}",
                                      name=f"mrb{chunk}")
                nc.sync.dma_start(mrb[:], m1out[chunk][:])
                dst = h_lo[1] if chunk == 0 else h_hi[1]
                for bb in range(nb):
                    nc.vector.tensor_scalar(
                        dst[:, bb, :], mrb[:, bb, :],
                        recip[:, boff + bb:boff + bb + 1], 0.0,
                        Alu.mult, Alu.max)

            for ly in range(NUM_LAYERS):
                sT_prev = emit_s(ly, 0)
                for b in range(B):
                    sT_next = emit_s(ly, b + 1) if b + 1 < B else None
                    emit_msum(ly, b, sT_prev)
                    sT_prev = sT_next
                    if ly == 0 and b == C1 - 1:
                        nc.gpsimd.collective_compute(
                            "AllReduce", Alu.add, replica_groups=rg,
                            ins=[m1in[0].opt()], outs=[m1out[0].opt()])
                        emit_readback(0)
                if ly == 0:
                    nc.gpsimd.collective_compute(
                        "AllReduce", Alu.add, replica_groups=rg,
                        ins=[m1in[1].opt()], outs=[m1out[1].opt()])
                    emit_readback(1)

            # layer 2: ReduceScatter -> each core gets its own batch summed
            nc.gpsimd.collective_compute(
                "ReduceScatter", Alu.add, replica_groups=rg,
                ins=[m2in.opt()], outs=[m2own.opt()])

            # keep PE warm during the ReduceScatter (HAM would re-throttle)
            for i in range(N_WARM):
                pw = mpsum.tile([128, D], f32, tag="mm", name="warmp")
                for j in range(2):
                    nc.tensor.matmul(pw[:], lhsT=identity[:], rhs=warm_sb[:],
                                     start=(j == 0), stop=(j == 1))
                nc.vector.tensor_copy(warm_sb[:, :16], pw[:, :16])

            mo = cpool.tile([128, D], f16, tag="mo")
            nc.sync.dma_start(mo[:], m2own[0])
            h_own = cpool.tile([128, D], f16, tag="hown")
            nc.vector.tensor_scalar(h_own[:], mo[:], recip_o[:], 0.0,
                                    Alu.mult, Alu.max)

            # -------- MLP on own batch --------------------------------------
            hT = cpool.tile([128, KC, 128], f16, tag="hT")
            pt = mpsum.tile([128, KC, 128], f16, tag="mm", name="ptr")
            for kc in range(KC):
                nc.tensor.transpose(pt[:, kc, :],
                                    h_own[:, kc * 128:(kc + 1) * 128],
                                    identity[:])
            nc.vector.tensor_copy(hT[:], pt[:])

            x1T = cpool.tile([128, KC, 128], f16, tag="x1T")
            px1 = mpsum.tile([128, KC, 128], f32, tag="mm", name="px1")
            for blk in range(KC):
                for kc in range(KC):
                    nc.tensor.matmul(
                        px1[:, blk, :],
                        lhsT=w0T_sb[:, kc, blk * 128:(blk + 1) * 128],
                        rhs=hT[:, kc, :],
                        start=(kc == 0), stop=(kc == KC - 1),
                    )
            for blk in range(KC):
                nc.vector.tensor_scalar(x1T[:, blk, :], px1[:, blk, :],
                                        b0_sb[:, blk:blk + 1], 0.0,
                                        Alu.add, Alu.max)

            x2 = cpool.tile([128, KC, 128], f32, tag="x2")
            px2 = mpsum.tile([128, KC, 128], f32, tag="mm", name="px2")
            for blk in range(KC):
                for kc in range(KC):
                    nc.tensor.matmul(
                        px2[:, blk, :],
                        lhsT=w1T_sb[:, kc, blk * 128:(blk + 1) * 128],
                        rhs=x1T[:, kc, :],
                        start=(kc == 0), stop=(kc == KC - 1),
                    )
            for blk in range(KC):
                nc.vector.tensor_scalar(x2[:, blk, :], px2[:, blk, :],
                                        b1_sb[:, blk:blk + 1], 0.0,
                                        Alu.add, Alu.max)

            for blk in range(KC):
                nc.sync.dma_start(out_e[blk], x2[:, blk, :])

    nc.compile()
    return nc


def _get_nc():
    if "nc" not in _CACHE:
        _CACHE["nc"] = _build_nc()
    return _CACHE["nc"]


def kernel(gcn_inputs, word_seq_len, adj_matrix, dep_label_matrix,
           w_params, mlp_w0, mlp_b0, mlp_w1, mlp_b1, **_unused):
    from concourse.bass_utils import run_bass_kernel_spmd

    gcn = np.asarray(gcn_inputs, dtype=np.float32)
    adj = np.asarray(adj_matrix, dtype=np.float32)
    lab = np.asarray(dep_label_matrix)
    w = np.asarray(w_params, dtype=np.float32)
    w0 = np.asarray(mlp_w0, dtype=np.float32)
    w1 = np.asarray(mlp_w1, dtype=np.float32)
    b0 = np.asarray(mlp_b0, dtype=np.float32)
    b1 = np.asarray(mlp_b1, dtype=np.float32)

    # [j, b, i] layouts for SBUF partition-major contiguous DMA
    adjT = np.ascontiguousarray(adj.transpose(2, 0, 1))
    labT = np.ascontiguousarray(lab.transpose(2, 0, 1)).astype(np.float32)
    adjR = np.ascontiguousarray(adj.transpose(1, 0, 2))
    gcnT = np.ascontiguousarray(gcn.transpose(1, 0, 2))
    # wT[kmod, l, kc, d] = w[l, d, kc*128+kmod]
    wT = w.transpose(0, 2, 1).reshape(L, KC, 128, D).transpose(2, 0, 1, 3)
    wT = np.ascontiguousarray(wT).astype(np.float16)
    w0T = np.ascontiguousarray(
        w0.T.reshape(KC, 128, D).transpose(1, 0, 2)).astype(np.float16)
    w1T = np.ascontiguousarray(
        w1.T.reshape(KC, 128, D).transpose(1, 0, 2)).astype(np.float16)
    b0r = np.ascontiguousarray(b0.reshape(KC, 128).T)   # [dmod, dblk]
    b1r = np.ascontiguousarray(b1.reshape(KC, 128).T)

    in_maps = []
    for c in range(NCORES):
        loff = np.tile(np.arange(LC * c, LC * (c + 1), dtype=np.float32),
                       (128, 1))
        in_maps.append({
            "gcn": gcnT,
            "adjT": adjT,
            "labT": labT,
            "adjR": adjR,
            "adjown": np.ascontiguousarray(adj[c]),
            "wT": np.ascontiguousarray(wT[:, LC * c:LC * (c + 1)]),
            "w0T": w0T,
            "w1T": w1T,
            "b0": b0r,
            "b1": b1r,
            "loff": loff,
        })

    nc = _get_nc()
    res = run_bass_kernel_spmd(nc, in_maps, list(range(NCORES)))

    out = np.empty((B, N, D), dtype=np.float32)
    for c in range(NCORES):
        arr = res.results[c]["out"]          # [dblk, dmod, i]
        out[c] = np.transpose(arr, (2, 0, 1)).reshape(N, D)
    return out
